# revision 18
# baseline (speedup 1.0000x reference)
"""Trainium2 Bass kernel for nn_CMR_59931973648949 (gnn_message_passing).

Contract: kernel(**inputs) takes FULL unsharded numpy inputs and returns the
FULL [16, 1024] output. Data-parallel over batch across 8 cores (2 samples
per core, weights replicated). All weights are host-packed partition-major
([128, F]) for max-bandwidth DMAs; the two local samples are batched through
the shared heavy matmuls (feat_v, q/u0/u1 projections); the big matmuls run
as float32r (full PE rate at free-dim >= 256).

Math per sample (refactored; see git history for derivation):
  scl[n] = mean(norm_w)/max(||visf[:,n]||,1e-12)   (folded into feat_v scale)
  feat_v = (visf.T * scl) @ W_v.T ; used only via feat_vT
  q/u0/u1 from node/relate reps with WnT=W_node.T/sqrt(DV),
      WA0/1=W_rel.T@W_e[:, :DV | DV:]/sqrt(DE)
  find = softmax(mask(q @ feat_vT)) * node_mask
  ea_r = sigmoid(A0[r,:] bcast + A1T[:,r]) * relation_mask
  g_findT = find.T-gather via GT (folds valid*relate_mask*onehot(obj))
  h[r,:] = g_find[r,:] @ ea_r ; find2T = findT + h.T @ ST (onehot(subj))
  fa = rowmax(find2T); fa /= max(max(fa),1); fa = fa*bm + (1-bm)*1e-7
  mem = visf @ fa ; out = mem @ W_out.T + b_out
"""

import numpy as np

import concourse.bass as bass
import concourse.tile as tile
from concourse import bacc, mybir
from concourse.bass_utils import run_bass_kernel_spmd

P = 128
B, K, R, N = 16, 12, 12, 64
DW, DV, DVIS, DE, DC = 512, 512, 2048, 512, 1024
NCORES = 8
S = B // NCORES  # samples per core = 2
N2 = S * N  # 128: both samples' boxes side by side
K2 = S * K  # 24

F32 = mybir.dt.float32
F32R = mybir.dt.float32r
BF16 = mybir.dt.bfloat16
F16 = mybir.dt.float16
F8E4 = mybir.dt.float8e4
USE_F32R = True
HALF = F16        # half dtype for weight DMAs (fp16: 10-bit mantissa)
WVT_BF16 = True   # feat_v matmul operands in half
WCAT_BF16 = True  # q/u0/u1 weight + reps in bf16
WOT_BF16 = True   # W_out matmul operands in half (fp16 keeps ~3e-4)
# W_out ships as two stacked e4m3 levels (base + scaled residual); the two
# matmul passes accumulate in psum and recover ~fp16 accuracy at half the
# DMA bytes. rhs (mem) stays fp16 (mixed-dtype matmul).
CBLK = DC // P    # 8 column blocks of 128 output channels

# smalls packing column offsets (per sample, [64, SMALLS_F])
_SM_RM = 0          # rmask      [64, 64]
_SM_BM = 64         # bmmul      [12, 64]
_SM_BA = 128        # bmadd      [12, 64]
_SM_GT = 192        # GT         [12, 12]
_SM_ST = 204        # ST         [12, 12]
_SM_NM = 216        # nmcol      [12, 1]
_SM_FM = 217        # famul      [1, 64]
_SM_FA = 281        # faadd      [1, 64]
SMALLS_F = 352

_cache = {}


def _pack(a):
    """[(o*128), F] row-major -> [128, o*F] partition-major."""
    o = a.shape[0] // P
    return np.ascontiguousarray(
        a.reshape(o, P, a.shape[1]).transpose(1, 0, 2).reshape(P, -1)
    )


def build_nc(bm_ones=False, nm_ones=False, rm_ones=False):
    nc = bacc.Bacc(num_devices=NCORES)

    FR = F32R if USE_F32R else F32
    d_visf = nc.declare_dram_parameter("visf16", [P, 16 * N2], HALF, isOutput=False)
    d_WvT = nc.declare_dram_parameter("WvT8", [P, 16 * DV], F8E4, isOutput=False)
    # wcat16 = nrepT2 | rrepT2 | I12 (fp16); wq8 = WnT | WA0 | WA1 (e4m3)
    WCATF = 2 * 4 * K2 + K
    d_wcat = nc.declare_dram_parameter("wcat16", [P, WCATF], HALF, isOutput=False)
    d_wq8 = nc.declare_dram_parameter("wq8", [P, 3 * 4 * DV], F8E4, isOutput=False)
    # W_out: two e4m3 levels (base, residual) stacked level-major
    d_WoT = nc.declare_dram_parameter("WoT8", [P, 2 * 16 * DC], F8E4, isOutput=False)
    # oscl: col0 = sa (out scale, replicated), col1 = sb/sa (residual rhs
    # scale, replicated), cols 2..2+CBLK = bias column-major [p, cb],
    # then tn*sv, t0*sv, t1*sv (query-weight x WvT dequant scales, replicated)
    NSCL = 2 + CBLK + 3
    d_oscl = nc.declare_dram_parameter("oscl", [P, NSCL], F32, isOutput=False)
    # resth = I128 [P, 128] | smalls [64, S*SMALLS_F]  (all fp16)
    d_rest = nc.declare_dram_parameter(
        "resth", [P, P + S * SMALLS_F], HALF, isOutput=False
    )
    d_out = nc.declare_dram_parameter("out", [S, DC], F16, isOutput=True)

    with tile.TileContext(nc) as tc:
        with (
            tc.tile_pool(name="singles", bufs=1) as singles,
            tc.tile_pool(name="ps", bufs=2) as ps,
            tc.tile_pool(name="psum", bufs=8, space="PSUM") as psum,
        ):
            # ---- DMAs on the critical path first (SP queue runs in order) ----
            visf2_mm = singles.tile([P, 16, N2], HALF)
            nc.sync.dma_start(
                out=visf2_mm[:], in_=d_visf[:].rearrange("p (o n) -> p o n", o=16)
            )
            rest_sb = singles.tile([P, P + S * SMALLS_F], HALF)
            nc.sync.dma_start(out=rest_sb[:], in_=d_rest[:])
            I128_sb = rest_sb[:, :P]
            smalls_sb = [
                rest_sb[:N, P + s * SMALLS_F : P + (s + 1) * SMALLS_F]
                for s in range(S)
            ]
            # reps + query weights land before the big W_v stream: the
            # q/u0/u1 gate is the chain-start bottleneck, feat_v is not
            oscl_sb = singles.tile([P, NSCL], F32)
            nc.sync.dma_start(out=oscl_sb[:], in_=d_oscl[:])
            wcatall_sb = singles.tile([P, WCATF], HALF)
            nc.sync.dma_start(out=wcatall_sb[:], in_=d_wcat[:])
            wq8_sb = singles.tile([P, 3, 4, DV], F8E4)
            nc.sync.dma_start(
                out=wq8_sb[:], in_=d_wq8[:].rearrange("p (t o d) -> p t o d", t=3, o=4)
            )
            WvT_sb = singles.tile([P, 16, DV], F8E4)
            for g in range(2):
                nc.sync.dma_start(
                    out=WvT_sb[:, 8 * g : 8 * g + 8, :],
                    in_=d_WvT[:, 8 * g * DV : 8 * (g + 1) * DV].rearrange(
                        "p (o d) -> p o d", o=8
                    ),
                )
            reps_sb = wcatall_sb[:, : 2 * 4 * K2].rearrange(
                "p (t o k) -> p t o k", t=2, o=4
            )
            I12h_sb = wcatall_sb[:K, 2 * 4 * K2 : 2 * 4 * K2 + K]
            WnT_sb = wq8_sb[:, 0]
            WA0_sb = wq8_sb[:, 1]
            WA1_sb = wq8_sb[:, 2]

            ones_col = singles.tile([P, 1], F32)
            nc.vector.memset(ones_col[:], 1.0)
            ones_1xP = singles.tile([1, P], HALF)
            nc.vector.memset(ones_1xP[:], 1.0)

            nrep2 = reps_sb[:, 0]  # [P, 4, 24]
            rrep2 = reps_sb[:, 1]
            HALF_SM = HALF if WCAT_BF16 else F32  # dtype of small attention mms

            # ---- shared: column norms via gram diagonal ----
            gram_ps = psum.tile([N2, N2], F32, tag="ps")
            for c in range(16):
                nc.tensor.matmul(
                    out=gram_ps[:],
                    lhsT=visf2_mm[:, c, :],
                    rhs=visf2_mm[:, c, :],
                    start=(c == 0),
                    stop=(c == 15),
                )
            gd_sb = singles.tile([N2, N2], F32)
            nc.vector.tensor_tensor(
                out=gd_sb[:], in0=gram_ps[:], in1=I128_sb[:],
                op=mybir.AluOpType.mult,
            )
            scl = singles.tile([N2, 1], F32)
            nc.vector.tensor_reduce(
                out=scl[:], in_=gd_sb[:], axis=mybir.AxisListType.X,
                op=mybir.AluOpType.add,
            )
            nc.scalar.sqrt(out=scl[:], in_=scl[:])
            nc.vector.tensor_scalar_max(out=scl[:], in0=scl[:], scalar1=1e-12)
            nc.vector.reciprocal(out=scl[:], in_=scl[:])

            # ---- shared: feat_v for both samples [n2, 512] ----
            featv_ps = psum.tile([N2, DV], F32, tag="ps")
            for c in range(16):
                nc.tensor.matmul(
                    out=featv_ps[:],
                    lhsT=visf2_mm[:, c, :],
                    rhs=WvT_sb[:, c, :],
                    start=(c == 0),
                    stop=(c == 15),
                )
            featv_sb = singles.tile([N2, DV], HALF)
            nc.vector.tensor_scalar_mul(out=featv_sb[:], in0=featv_ps[:], scalar1=scl[:])
            ftT2_ps = psum.tile([P, 4, N2], F32, tag="ps")
            for c in range(4):
                nc.tensor.matmul(
                    out=ftT2_ps[:, c, :],
                    lhsT=featv_sb[:, P * c : P * (c + 1)],
                    rhs=I128_sb[:],
                    start=(c == 0),
                    stop=(c == 3),
                )
            ftT2_sb = singles.tile([P, 4, N2], HALF_SM)
            nc.vector.tensor_copy(out=ftT2_sb[:], in_=ftT2_ps[:])

            # ---- shared: qT/u0T/u1T for both samples [d, 24] ----
            # dequant by tn*sv / t0*sv / t1*sv (ptr scales) on the psum drain
            def lin_T(w_sb, x_ap, name, scl_idx):
                out_ps = psum.tile([P, 4, K2], F32, tag="ps", name=name + "_ps")
                for dc in range(4):
                    for wc in range(4):
                        nc.tensor.matmul(
                            out=out_ps[:, dc, :],
                            lhsT=w_sb[:, wc, P * dc : P * (dc + 1)],
                            rhs=x_ap[:, wc, :],
                            start=(dc == 0 and wc == 0),
                            stop=(dc == 3 and wc == 3),
                        )
                out_sb = singles.tile([P, 4, K2], HALF_SM, name=name)
                nc.vector.tensor_scalar_mul(
                    out=out_sb[:], in0=out_ps[:],
                    scalar1=oscl_sb[:, scl_idx : scl_idx + 1],
                )
                return out_sb

            qT2_sb = lin_T(WnT_sb, nrep2, "qT2", 2 + CBLK)
            u0T2_sb = lin_T(WA0_sb, rrep2, "u0T2", 2 + CBLK + 1)
            u1T2_sb = lin_T(WA1_sb, rrep2, "u1T2", 2 + CBLK + 2)

            # ---- W_out weight stream (runs during the sample chains) ----
            WoT_sb = singles.tile([P, 2, 16, DC], F8E4)
            for lv in range(2):
                nc.sync.dma_start(
                    out=WoT_sb[:, lv],
                    in_=d_WoT[:, lv * 16 * DC : (lv + 1) * 16 * DC].rearrange(
                        "p (o d) -> p o d", o=16
                    ),
                )
            mem2r_sb = singles.tile([P, 16, S], HALF)
            mem2rB_sb = singles.tile([P, 16, S], HALF)

            # ---- per-sample pipeline, stages interleaved across samples ----
            st = [dict() for _ in range(S)]
            for s in range(S):
                sm = smalls_sb[s]
                st[s]["rmask"] = sm[:, _SM_RM : _SM_RM + N]
                st[s]["bmmul"] = sm[:K, _SM_BM : _SM_BM + N]
                st[s]["bmadd"] = sm[:K, _SM_BA : _SM_BA + N]
                st[s]["GTm"] = sm[:K, _SM_GT : _SM_GT + R]
                st[s]["STm"] = sm[:R, _SM_ST : _SM_ST + K]
                st[s]["nmcol"] = sm[:K, _SM_NM : _SM_NM + 1]
                st[s]["famul"] = sm[:1, _SM_FM : _SM_FM + N]
                st[s]["faadd"] = sm[:1, _SM_FA : _SM_FA + N]
                st[s]["ks"] = slice(K * s, K * (s + 1))
                st[s]["ns"] = slice(N * s, N * (s + 1))

            def stage_softmax(s):
                d = st[s]
                logits_ps = psum.tile([K, N], F32, tag="ps", name=f"lg_ps{s}")
                for c in range(4):
                    nc.tensor.matmul(
                        out=logits_ps[:],
                        lhsT=qT2_sb[:, c, d["ks"]],
                        rhs=ftT2_sb[:, c, d["ns"]],
                        start=(c == 0),
                        stop=(c == 3),
                    )
                if bm_ones:
                    lg_sb = logits_ps
                else:
                    lg_sb = ps.tile([K, N], F32, name=f"lg{s}", tag=f"lg{s}")
                    nc.vector.tensor_tensor(
                        out=lg_sb[:], in0=logits_ps[:], in1=d["bmmul"],
                        op=mybir.AluOpType.mult,
                    )
                    nc.vector.tensor_tensor(
                        out=lg_sb[:], in0=lg_sb[:], in1=d["bmadd"],
                        op=mybir.AluOpType.add,
                    )
                nmx = ps.tile([K, 1], F32, name=f"nmx{s}", tag=f"nmx{s}")
                nc.vector.tensor_reduce(
                    out=nmx[:], in_=lg_sb[:], axis=mybir.AxisListType.X,
                    op=mybir.AluOpType.max, negate=True,
                )
                e_sb = ps.tile([K, N], F32, name=f"e{s}", tag=f"e{s}")
                ssum = ps.tile([K, 1], F32, name=f"ss{s}", tag=f"ss{s}")
                nc.scalar.activation(
                    out=e_sb[:], in_=lg_sb[:],
                    func=mybir.ActivationFunctionType.Exp,
                    bias=nmx[:], scale=1.0, accum_out=ssum[:],
                )
                rs = ps.tile([K, 1], F32, name=f"rs{s}", tag=f"rs{s}")
                nc.vector.reciprocal(out=rs[:], in_=ssum[:])
                if not nm_ones:
                    nc.vector.tensor_tensor(
                        out=rs[:], in0=rs[:], in1=d["nmcol"], op=mybir.AluOpType.mult
                    )
                find_sb = ps.tile([K, N], HALF, name=f"find{s}", tag=f"find{s}")
                nc.vector.tensor_scalar_mul(out=find_sb[:], in0=e_sb[:], scalar1=rs[:])
                d["find"] = find_sb

            def stage_proj(s):
                d = st[s]
                find_sb = d["find"]
                gfT_ps = psum.tile([N, R], F32, tag="ps", name=f"gfT_ps{s}")
                nc.tensor.matmul(
                    out=gfT_ps[:], lhsT=find_sb[:], rhs=d["GTm"], start=True, stop=True
                )
                gfT_sb = ps.tile([N, R], HALF, name=f"gfT{s}", tag=f"gfT{s}")
                nc.scalar.copy(out=gfT_sb[:], in_=gfT_ps[:])
                d["gfT"] = gfT_sb
                f2T_ps = psum.tile([N, K], F32, tag="ps", name=f"f2T_ps{s}")
                nc.tensor.matmul(
                    out=f2T_ps[:], lhsT=find_sb[:], rhs=I128_sb[:K, :K],
                    start=True, stop=False,
                )
                d["f2T_ps"] = f2T_ps
                A0_ps = psum.tile([R, N], F32, tag="ps", name=f"A0_ps{s}")
                for c in range(4):
                    nc.tensor.matmul(
                        out=A0_ps[:], lhsT=u0T2_sb[:, c, d["ks"]],
                        rhs=ftT2_sb[:, c, d["ns"]],
                        start=(c == 0), stop=(c == 3),
                    )
                A0_sb = ps.tile([R, N], HALF_SM, name=f"A0{s}", tag=f"A0{s}")
                nc.scalar.copy(out=A0_sb[:], in_=A0_ps[:])
                d["A0"] = A0_sb
                A1_ps = psum.tile([R, N], F32, tag="ps", name=f"A1_ps{s}")
                for c in range(4):
                    nc.tensor.matmul(
                        out=A1_ps[:], lhsT=u1T2_sb[:, c, d["ks"]],
                        rhs=ftT2_sb[:, c, d["ns"]],
                        start=(c == 0), stop=(c == 3),
                    )
                A1_sb = ps.tile([R, N], HALF_SM, name=f"A1{s}", tag=f"A1{s}")
                nc.scalar.copy(out=A1_sb[:], in_=A1_ps[:])
                d["A1"] = A1_sb

            def stage_edge(s):
                d = st[s]
                ea_all = ps.tile([N, R, N], HALF, name=f"ea{s}", tag=f"ea{s}")
                GR = R // 2
                for g in range(2):
                    Bg = psum.tile([N, GR, N], F32, tag="ps", name=f"B6_{s}{g}")
                    for i in range(GR):
                        r = GR * g + i
                        sel = I12h_sb[:, r : r + 1].to_broadcast([K, N])
                        nc.tensor.matmul(
                            out=Bg[:, i, :], lhsT=sel, rhs=d["A0"][:],
                            start=(i == 0), stop=False,
                        )
                        nc.tensor.matmul(
                            out=Bg[:, i, :], lhsT=d["A1"][:], rhs=sel,
                            start=False, stop=(i == GR - 1),
                        )
                    nc.scalar.activation(
                        out=ea_all[:, GR * g : GR * (g + 1), :], in_=Bg[:],
                        func=mybir.ActivationFunctionType.Sigmoid,
                    )
                if not rm_ones:
                    nc.vector.tensor_tensor(
                        out=ea_all[:],
                        in0=ea_all[:],
                        in1=d["rmask"][:, None, :].to_broadcast([N, R, N]),
                        op=mybir.AluOpType.mult,
                    )
                d["ea"] = ea_all

            def stage_h(s):
                d = st[s]
                hT_ps = psum.tile([N, R], F32, tag="ps", name=f"hT_ps{s}")
                for r in range(R):
                    nc.tensor.matmul(
                        out=hT_ps[:, r : r + 1],
                        lhsT=d["ea"][:, r, :],
                        rhs=d["gfT"][:, r : r + 1],
                        start=(r == 0),
                        stop=(r == R - 1),
                    )
                hT_sb = ps.tile([N, R], HALF, name=f"hT{s}", tag=f"hT{s}")
                nc.scalar.copy(out=hT_sb[:], in_=hT_ps[:])
                h_ps = psum.tile([R, N], F32, tag="ps", name=f"h_ps{s}")
                nc.tensor.matmul(
                    out=h_ps[:], lhsT=hT_sb[:], rhs=I128_sb[:N, :N],
                    start=True, stop=True,
                )
                h_sb = ps.tile([R, N], HALF, name=f"h{s}", tag=f"h{s}")
                nc.scalar.copy(out=h_sb[:], in_=h_ps[:])
                nc.tensor.matmul(
                    out=d["f2T_ps"][:], lhsT=h_sb[:], rhs=d["STm"],
                    start=False, stop=True,
                )

            def stage_mem(s):
                d = st[s]
                fa_sb = ps.tile([N, 1], HALF, name=f"fa{s}", tag=f"fa{s}")
                nc.vector.tensor_reduce(
                    out=fa_sb[:], in_=d["f2T_ps"][:], axis=mybir.AxisListType.X,
                    op=mybir.AluOpType.max,
                )
                faT_ps = psum.tile([1, N], F32, tag="ps", name=f"faT_ps{s}")
                nc.tensor.matmul(
                    out=faT_ps[:], lhsT=fa_sb[:], rhs=I128_sb[:N, :N],
                    start=True, stop=True,
                )
                nr = ps.tile([1, 1], F32, name=f"nr{s}", tag=f"nr{s}")
                nc.vector.tensor_reduce(
                    out=nr[:], in_=faT_ps[:], axis=mybir.AxisListType.X,
                    op=mybir.AluOpType.max,
                )
                nc.vector.tensor_scalar_max(out=nr[:], in0=nr[:], scalar1=1.0)
                nc.vector.reciprocal(out=nr[:], in_=nr[:])
                faT_sb = ps.tile([1, N], HALF, name=f"faT{s}", tag=f"faT{s}")
                nc.vector.tensor_scalar_mul(out=faT_sb[:], in0=faT_ps[:], scalar1=nr[:])
                if not bm_ones:
                    nc.vector.tensor_tensor(
                        out=faT_sb[:], in0=faT_sb[:], in1=d["famul"],
                        op=mybir.AluOpType.mult,
                    )
                    nc.vector.tensor_tensor(
                        out=faT_sb[:], in0=faT_sb[:], in1=d["faadd"],
                        op=mybir.AluOpType.add,
                    )
                fabc_ps = psum.tile([P, N], F32, tag="ps", name=f"fabc_ps{s}")
                nc.tensor.matmul(
                    out=fabc_ps[:], lhsT=ones_1xP[:], rhs=faT_sb[:],
                    start=True, stop=True,
                )
                fabc_sb = ps.tile([P, N], HALF, name=f"fabc{s}", tag=f"fabc{s}")
                nc.scalar.copy(out=fabc_sb[:], in_=fabc_ps[:])
                wtmp = ps.tile([P, 16, N], HALF, tag="wtmp")
                nc.vector.tensor_tensor(
                    out=wtmp[:],
                    in0=visf2_mm[:, :, d["ns"]],
                    in1=fabc_sb[:, None, :].to_broadcast([P, 16, N]),
                    op=mybir.AluOpType.mult,
                )
                # reduce straight into the fp16 W_out operand (same rounding
                # the cast copy applied; skips one critical-path step)
                with nc.allow_low_precision("fp16 mem rounding, matches cast"):
                    nc.vector.tensor_reduce(
                        out=mem2r_sb[:, :, s], in_=wtmp[:],
                        axis=mybir.AxisListType.X,
                        op=mybir.AluOpType.add,
                    )

            def stage_wout_all():
                # outT[c, s] = sum_v WoT8a[v, c]*mem[v, s] + WoT8b[v, c]*memB[v, s]
                # weights stationary (Ldweights is ~free), 2-wide moving rhs.
                # Per 128-channel block: 32 accumulating matmuls, then a fused
                # scale(sa)+bias drain; finally transpose to [S, DC] and DMA.
                nc.vector.tensor_scalar_mul(
                    out=mem2rB_sb[:], in0=mem2r_sb[:], scalar1=oscl_sb[:, 1:2]
                )
                outT_sb = singles.tile([P, CBLK, S], HALF)
                for cb in range(CBLK):
                    o_ps = psum.tile([P, S], F32, tag="ps", name=f"o{cb}")
                    for c in range(16):
                        nc.tensor.matmul(
                            out=o_ps[:],
                            lhsT=WoT_sb[:, 0, c, P * cb : P * (cb + 1)],
                            rhs=mem2r_sb[:, c, :],
                            start=(c == 0), stop=False,
                        )
                    for c in range(16):
                        nc.tensor.matmul(
                            out=o_ps[:],
                            lhsT=WoT_sb[:, 1, c, P * cb : P * (cb + 1)],
                            rhs=mem2rB_sb[:, c, :],
                            start=False, stop=(c == 15),
                        )
                    nc.vector.tensor_scalar(
                        out=outT_sb[:, cb, :], in0=o_ps[:],
                        scalar1=oscl_sb[:, 0:1],
                        scalar2=oscl_sb[:, 2 + cb : 3 + cb],
                        op0=mybir.AluOpType.mult, op1=mybir.AluOpType.add,
                    )
                oT_ps = psum.tile([S, CBLK, P], HALF, tag="ps", name="oT")
                for cb in range(CBLK):
                    nc.tensor.transpose(
                        out=oT_ps[:, cb, :], in_=outT_sb[:, cb, :],
                        identity=I128_sb[:],
                    )
                outcat = singles.tile([S, DC], HALF)
                nc.scalar.copy(out=outcat[:], in_=oT_ps[:].rearrange("s c p -> s (c p)"))
                nc.sync.dma_start(out=d_out[:], in_=outcat[:])

            for stage in (
                stage_softmax, stage_proj, stage_edge, stage_h, stage_mem
            ):
                for s in range(S):
                    stage(s)
            stage_wout_all()

    nc.finalize()
    return nc


def _host_prep(inputs):
    node_rep = np.asarray(inputs["node_rep"], np.float32)
    relate_rep = np.asarray(inputs["relate_rep"], np.float32)
    relate_os = np.asarray(inputs["relate_os"])
    relate_mask = np.asarray(inputs["relate_mask"], np.float32)
    vision_feat = np.asarray(inputs["vision_feat"], np.float32)
    relation_mask = np.asarray(inputs["relation_mask"], np.float32)
    box_mask = np.asarray(inputs["box_mask"], np.float32)
    node_mask = np.asarray(inputs["node_mask"], np.float32)
    norm_w = np.asarray(inputs["norm_w"], np.float32)
    W_v = np.asarray(inputs["W_v"], np.float32)
    W_e = np.asarray(inputs["W_e"], np.float32)
    W_node = np.asarray(inputs["W_node"], np.float32)
    W_rel = np.asarray(inputs["W_rel"], np.float32)
    W_out = np.asarray(inputs["W_out"], np.float32)
    b_out = np.asarray(inputs["b_out"], np.float32)

    s_mean = np.float32(np.mean(norm_w))
    WvT = (W_v.T * s_mean).astype(np.float32)
    WnT = (W_node.T / np.float32(np.sqrt(DV))).astype(np.float32)
    WA0 = (W_rel.T @ W_e[:, :DV] / np.float32(np.sqrt(DE))).astype(np.float32)
    WA1 = (W_rel.T @ W_e[:, DV:] / np.float32(np.sqrt(DE))).astype(np.float32)
    WoT = np.ascontiguousarray(W_out.T)

    # two-level e4m3 quantization of WoT (base + residual), per-tensor scales
    import ml_dtypes

    E4 = ml_dtypes.float8_e4m3
    fmax8 = np.float32(ml_dtypes.finfo(E4).max)
    sa = np.float32(max(np.max(np.abs(WoT)), 1e-30) / fmax8)
    W8a = (WoT / sa).astype(E4)
    resid = WoT - W8a.astype(np.float32) * sa
    sb = np.float32(max(np.max(np.abs(resid)), 1e-30) / fmax8)
    W8b = (resid / sb).astype(E4)

    # single-level e4m3 for W_v^T and the query weights; dequant scales are
    # folded into the q/u0/u1 psum drains (logits are linear in each)
    def q8(a):
        s = np.float32(max(np.max(np.abs(a)), 1e-30) / fmax8)
        return (a / s).astype(E4), s

    Wv8, sv = q8(WvT)
    Wn8, tn = q8(WnT)
    WA08, t0 = q8(WA0)
    WA18, t1 = q8(WA1)

    subj = relate_os[..., 1].astype(np.int64)
    obj = relate_os[..., 0].astype(np.int64)
    valid = (subj != -1).astype(np.float32)
    obj_c = np.clip(obj, 0, K - 1)
    subj_c = np.clip(subj, 0, K - 1)
    G = np.zeros((B, R, K), np.float32)
    STm = np.zeros((B, R, K), np.float32)
    bi = np.arange(B)[:, None]
    ri = np.arange(R)[None, :]
    G[bi, ri, obj_c] = valid * relate_mask
    STm[bi, ri, subj_c] = 1.0

    bmmul = (box_mask > 0).astype(np.float32)
    bmadd = (bmmul - 1.0) * np.float32(6e4)  # fp16-safe large negative
    famul = box_mask
    faadd = (1.0 - box_mask) * np.float32(1e-7)

    WvT_p = _pack(Wv8.astype(np.float32)).astype(E4)
    WoT8_p = np.concatenate(
        [_pack(W8a.astype(np.float32)), _pack(W8b.astype(np.float32))], axis=1
    ).astype(E4)
    wq8_p = np.concatenate(
        [
            _pack(Wn8.astype(np.float32)),
            _pack(WA08.astype(np.float32)),
            _pack(WA18.astype(np.float32)),
        ],
        axis=1,
    ).astype(E4)
    I128 = np.eye(P, dtype=np.float32)
    # oscl: [sa | sb/sa | bias column-major | tn*sv | t0*sv | t1*sv]
    oscl = np.zeros((P, 2 + CBLK + 3), np.float32)
    oscl[:, 0] = sa
    oscl[:, 1] = sb / sa
    oscl[:, 2 : 2 + CBLK] = b_out.reshape(CBLK, P).T
    oscl[:, 2 + CBLK] = tn * sv
    oscl[:, 2 + CBLK + 1] = t0 * sv
    oscl[:, 2 + CBLK + 2] = t1 * sv

    def smalls_for(b):
        sm = np.zeros((N, SMALLS_F), np.float32)
        sm[:, _SM_RM : _SM_RM + N] = relation_mask[b]
        sm[:K, _SM_BM : _SM_BM + N] = bmmul[b][None, :]
        sm[:K, _SM_BA : _SM_BA + N] = bmadd[b][None, :]
        sm[:K, _SM_GT : _SM_GT + R] = G[b].T
        sm[:R, _SM_ST : _SM_ST + K] = STm[b]
        sm[:K, _SM_NM] = node_mask[b]
        sm[0, _SM_FM : _SM_FM + N] = famul[b]
        sm[0, _SM_FA : _SM_FA + N] = faadd[b]
        return sm

    in_maps = []
    for c in range(NCORES):
        b0 = S * c
        visf2 = np.concatenate(
            [_pack(vision_feat[b]).reshape(P, 16, N) for b in range(b0, b0 + S)],
            axis=2,
        ).reshape(P, -1)
        nrep2 = np.concatenate(
            [
                _pack(np.ascontiguousarray(node_rep[b].T)).reshape(P, 4, K)
                for b in range(b0, b0 + S)
            ],
            axis=2,
        ).reshape(P, -1)
        rrep2 = np.concatenate(
            [
                _pack(np.ascontiguousarray(relate_rep[b].T)).reshape(P, 4, R)
                for b in range(b0, b0 + S)
            ],
            axis=2,
        ).reshape(P, -1)
        I12blk = np.zeros((P, K), np.float32)
        I12blk[:K, :K] = np.eye(K, dtype=np.float32)
        wcat_full = np.ascontiguousarray(
            np.concatenate([nrep2, rrep2, I12blk], axis=1)
        )
        smalls2 = np.concatenate(
            [smalls_for(b) for b in range(b0, b0 + S)], axis=1
        )  # [64, S*SMALLS_F]
        rest32 = np.zeros((P, P + S * SMALLS_F), np.float32)
        rest32[:, :P] = I128
        rest32[:N, P:] = smalls2
        m = {
            "visf16": np.ascontiguousarray(visf2).astype(np.float16),
            "WvT8": WvT_p,
            "wcat16": wcat_full.astype(np.float16),
            "wq8": wq8_p,
            "WoT8": WoT8_p,
            "oscl": oscl,
            "resth": rest32.astype(np.float16),
        }
        in_maps.append(m)
    return in_maps


def kernel(**inputs) -> np.ndarray:
    bm_ones = bool(np.all(np.asarray(inputs["box_mask"]) == 1.0))
    nm_ones = bool(np.all(np.asarray(inputs["node_mask"]) == 1.0))
    rm_ones = bool(np.all(np.asarray(inputs["relation_mask"]) == 1.0))
    key = ("nc", bm_ones, nm_ones, rm_ones)
    if key not in _cache:
        _cache[key] = build_nc(bm_ones, nm_ones, rm_ones)
    nc = _cache[key]
    in_maps = _host_prep(inputs)
    res = run_bass_kernel_spmd(nc, in_maps, core_ids=list(range(NCORES)))
    outs = [np.asarray(res.results[c]["out"], np.float32) for c in range(NCORES)]
    return np.concatenate(outs, axis=0)



# revision 37
# speedup vs baseline: 1.4792x; 1.4792x over previous
"""Trainium2 Bass kernel for nn_CMR_59931973648949 (gnn_message_passing).

Contract: kernel(**inputs) takes FULL unsharded numpy inputs and returns the
FULL [16, 1024] output. Data-parallel over batch across 8 cores (2 samples
per core, weights replicated). Weights are host-packed partition-major
([128, F]); W_v/W_node/W_rel-fused weights ship as e4m3 (per-tensor scales
folded into the q/u0/u1 psum drains), W_out as fp16 in two column-halves.

Math per sample (refactored):
  scl[n] = mean(norm_w)/max(||visf[:,n]||,1e-12)   (applied on feat_v drain)
  feat_v = visf.T @ W_v.T * scl ; used via its transpose ftT2
  q/u0/u1 from node/relate reps with WnT=W_node.T/sqrt(DV),
      WA0/1=W_rel.T@W_e[:, :DV | DV:]/sqrt(DE)  (e4m3, dequant on drain;
      A0/A1 additionally halved for the tanh form of sigmoid)
  find = softmax(mask(q @ feat_vT)) * node_mask
  ea_r = tanh((A0[r,:] bcast + A1T[:,r])/2)  [sigmoid = 0.5 tanh + 0.5,
      the affine is folded into the h-stage drain via gs[r]]
  g_findT = find.T-gather via GT (folds valid*relate_mask*onehot(obj))
  h[r,:] = g_find[r,:] @ ea_r ; find2T = findT + h.T @ ST (onehot(subj))
  fa = rowmax(find2T); fa /= max(max(fa),1); fa = fa*bm + (1-bm)*1e-7
  mem = visf @ fa ; out = mem @ W_out.T + b_out

Both samples are batched on the partition axis everywhere (rows 0-11/32-43
for 12-row tensors via tile_position col 32, rows 0-63/64-127 for 64-row
tensors via col 64); diagonal blocks of I128 provide identities at matching
partition bases. The two mem reductions run on DVE and Pool concurrently.
"""

import numpy as np

import concourse.bass as bass
import concourse.tile as tile
from concourse import bacc, mybir
from concourse.bass_utils import run_bass_kernel_spmd

P = 128
B, K, R, N = 16, 12, 12, 64
DW, DV, DVIS, DE, DC = 512, 512, 2048, 512, 1024
NCORES = 8
S = B // NCORES  # samples per core = 2
N2 = S * N  # 128: both samples' boxes side by side
K2 = S * K  # 24
K44 = 32 + K  # batched 12-row tensors: s0 rows 0..12, s1 rows 32..44

F32 = mybir.dt.float32
F16 = mybir.dt.float16
F8E4 = mybir.dt.float8e4
HALF = F16
CBLK = DC // P  # 8 column blocks of 128 output channels

# rest (fp16) column layout: shared masks then per-sample block.
# bm masks are [44, 128]: they also mask out the wrong-sample column half
# of the batched [44, 128] logits (rows 0-11 live in cols 0-63, rows 32-43
# in cols 64-127), so the batched softmax reduces correctly over 128 cols.
_SH_BM = 0            # bm2mul [44, 128]
_SH_BA = 128          # bm2add [44, 128]
_SH_NM = 256          # nm2col [44, 1]
_SH_GS = 257          # gs2 [128, 12] (0.5 * sum_k nm[k] G[r,k], tanh affine,
#                       pre-broadcast down partitions; s0 rows 0-63, s1 64-127)
_SH_RM = 269          # rmask2 [128, 64]
SHARED_F = 333
_SM_GT = 0            # GT [12@32s, 12]
_SM_ST = 12           # ST [12@32s, 12]
_SM_FM = 24           # famul [1, 64]
_SM_FA = 88           # faadd [1, 64]
SMALLS_F = 152
REST_F = SHARED_F + S * SMALLS_F

# oscl (f32) columns: dequant scales then bias
_OS_QN = 0            # tn*sv (WnT dequant x WvT dequant)
_OS_Q0 = 1            # t0*sv*0.5
_OS_Q1 = 2            # t1*sv*0.5
_OS_BIAS = 3          # bias [128, CBLK] column-major
NSCL = 3 + CBLK

KP = K44  # reps padded to the 44-col batched layout (junk cols are zero)
WCAT_F = P + 2 * 4 * KP  # I128 | nrepT2 | rrepT2

_cache = {}


def _pack(a):
    """[(o*128), F] row-major -> [128, o*F] partition-major."""
    o = a.shape[0] // P
    return np.ascontiguousarray(
        a.reshape(o, P, a.shape[1]).transpose(1, 0, 2).reshape(P, -1)
    )


def build_nc(bm_ones=False, nm_ones=False, rm_ones=False, bz=False):
    nc = bacc.Bacc(num_devices=NCORES)

    d_visf = nc.declare_dram_parameter("visf16", [P, 16 * N2], HALF, isOutput=False)
    d_wcat = nc.declare_dram_parameter("wcat16", [P, WCAT_F], HALF, isOutput=False)
    d_WvT = nc.declare_dram_parameter("WvT8", [P, 16 * DV], F8E4, isOutput=False)
    d_wq8 = nc.declare_dram_parameter("wq8", [P, 3 * 4 * DV], F8E4, isOutput=False)
    d_rest = nc.declare_dram_parameter("resth", [P, REST_F], HALF, isOutput=False)
    d_oscl = nc.declare_dram_parameter("oscl", [P, NSCL], F32, isOutput=False)
    # W_out fp16, packed as two 512-channel halves (cb-half-major)
    d_WoT = nc.declare_dram_parameter("WoT16", [P, 2 * 16 * 512], HALF, isOutput=False)
    d_out = nc.declare_dram_parameter("out", [S, DC], F16, isOutput=True)

    TANH = rm_ones  # tanh form needs ea scale folded into h; rmask breaks it

    with tile.TileContext(nc) as tc:
        with (
            tc.tile_pool(name="singles", bufs=1) as singles,
            tc.tile_pool(name="ps", bufs=2) as ps,
            tc.tile_pool(name="psum", bufs=8, space="PSUM") as psum,
        ):
            # ---- DMA stream (SP queue runs in order) ----
            visf2_mm = singles.tile([P, 16, N2], HALF)
            nc.sync.dma_start(
                out=visf2_mm[:], in_=d_visf[:].rearrange("p (o n) -> p o n", o=16)
            )
            wcat_sb = singles.tile([P, WCAT_F], HALF)
            nc.sync.dma_start(out=wcat_sb[:], in_=d_wcat[:])
            I128_sb = wcat_sb[:, :P]
            reps_sb = wcat_sb[:, P:].rearrange("p (t o k) -> p t o k", t=2, o=4)
            # [P, 4, 44] each; cols 0-11 sample 0, 32-43 sample 1, rest zero
            nrep2 = reps_sb[:, 0]  # [P, 4, 24]
            rrep2 = reps_sb[:, 1]
            WvT_sb = singles.tile([P, 16, DV], F8E4)
            for g in range(2):
                nc.sync.dma_start(
                    out=WvT_sb[:, 8 * g : 8 * g + 8, :],
                    in_=d_WvT[:, 8 * g * DV : 8 * (g + 1) * DV].rearrange(
                        "p (o d) -> p o d", o=8
                    ),
                )
            wq8_sb = singles.tile([P, 3, 4, DV], F8E4)
            nc.sync.dma_start(
                out=wq8_sb[:], in_=d_wq8[:].rearrange("p (t o d) -> p t o d", t=3, o=4)
            )
            rest_sb = singles.tile([P, REST_F], HALF)
            nc.sync.dma_start(out=rest_sb[:], in_=d_rest[:])
            oscl_sb = singles.tile([P, NSCL], F32)
            nc.sync.dma_start(out=oscl_sb[:], in_=d_oscl[:])
            WoT_sb = singles.tile([P, 2, 16, 512], HALF)
            for h in range(2):
                nc.sync.dma_start(
                    out=WoT_sb[:, h],
                    in_=d_WoT[:, h * 16 * 512 : (h + 1) * 16 * 512].rearrange(
                        "p (o d) -> p o d", o=16
                    ),
                )

            ones_1xP = singles.tile([1, P], HALF)
            nc.vector.memset(ones_1xP[:], 1.0)

            bm2mul = rest_sb[:K44, _SH_BM : _SH_BM + N2]
            bm2add = rest_sb[:K44, _SH_BA : _SH_BA + N2]
            nm2col = rest_sb[:K44, _SH_NM : _SH_NM + 1]
            gs2 = rest_sb[:, _SH_GS : _SH_GS + R]
            rmask2 = rest_sb[:, _SH_RM : _SH_RM + N]
            sm = [rest_sb[:, SHARED_F + s * SMALLS_F :] for s in range(S)]

            def GTm(s):
                return sm[s][32 * s : 32 * s + K, _SM_GT : _SM_GT + R]

            def STm(s):
                return sm[s][32 * s : 32 * s + R, _SM_ST : _SM_ST + K]

            # ---- gram -> column norms -> scl (1/||.||, s_mean folded in Wv) ----
            gram_ps = psum.tile([N2, N2], F32, tag="ps")
            for c in range(16):
                nc.tensor.matmul(
                    out=gram_ps[:],
                    lhsT=visf2_mm[:, c, :],
                    rhs=visf2_mm[:, c, :],
                    start=(c == 0),
                    stop=(c == 15),
                )

            # ---- feat_v for both samples [n2, 512] (drain applies scl) ----
            featv_ps = psum.tile([N2, DV], F32, tag="ps")
            for c in range(16):
                nc.tensor.matmul(
                    out=featv_ps[:],
                    lhsT=visf2_mm[:, c, :],
                    rhs=WvT_sb[:, c, :],
                    start=(c == 0),
                    stop=(c == 15),
                )

            # scl chain (DVE+Act) — emitted before lin_T so the in-order DVE
            # queue reaches it as soon as the gram psum stops
            gd_sb = ps.tile([N2, N2], F32, tag="gd")
            nc.vector.tensor_tensor(
                out=gd_sb[:], in0=gram_ps[:], in1=I128_sb[:],
                op=mybir.AluOpType.mult,
            )
            scl = singles.tile([N2, 1], F32)
            nc.vector.tensor_reduce(
                out=scl[:], in_=gd_sb[:], axis=mybir.AxisListType.X,
                op=mybir.AluOpType.add,
            )
            nc.scalar.sqrt(out=scl[:], in_=scl[:])
            nc.vector.tensor_scalar_max(out=scl[:], in0=scl[:], scalar1=1e-12)
            nc.vector.reciprocal(out=scl[:], in_=scl[:])
            featv_sb = singles.tile([N2, DV], HALF)
            nc.vector.tensor_scalar_mul(out=featv_sb[:], in0=featv_ps[:], scalar1=scl[:])

            # ---- qT/u0T/u1T for both samples [d, 24], e4m3 weights ----
            def lin_T(w_sb, x_ap, name, scl_idx):
                out_ps = psum.tile([P, 4, KP], F32, tag="ps", name=name + "_ps")
                for dc in range(4):
                    for wc in range(4):
                        nc.tensor.matmul(
                            out=out_ps[:, dc, :],
                            lhsT=w_sb[:, wc, P * dc : P * (dc + 1)],
                            rhs=x_ap[:, wc, :],
                            start=(dc == 0 and wc == 0),
                            stop=(dc == 3 and wc == 3),
                        )
                out_sb = singles.tile([P, 4, KP], HALF, name=name)
                nc.vector.tensor_scalar_mul(
                    out=out_sb[:], in0=out_ps[:],
                    scalar1=oscl_sb[:, scl_idx : scl_idx + 1],
                )
                return out_sb

            qT2_sb = lin_T(wq8_sb[:, 0], nrep2, "qT2", _OS_QN)
            u0T2_sb = lin_T(wq8_sb[:, 1], rrep2, "u0T2", _OS_Q0)
            u1T2_sb = lin_T(wq8_sb[:, 2], rrep2, "u1T2", _OS_Q1)

            ftT2_ps = psum.tile([P, 4, N2], F32, tag="ps")
            for c in range(4):
                nc.tensor.matmul(
                    out=ftT2_ps[:, c, :],
                    lhsT=featv_sb[:, P * c : P * (c + 1)],
                    rhs=I128_sb[:],
                    start=(c == 0),
                    stop=(c == 3),
                )
            ftT2_sb = singles.tile([P, 4, N2], HALF)
            nc.gpsimd.tensor_copy(out=ftT2_sb[:], in_=ftT2_ps[:])

            # ---- batched attention: logits / A0 / A1 [44, 64] ----
            def att_mm(qsb, name):
                # [44, 128] in one region: junk lhsT cols are zero, each
                # sample's valid block is rows 32s..32s+12 x cols 64s..64s+64
                out_ps = psum.tile([K44, N2], F32, tag="ps", name=name)
                for c in range(4):
                    nc.tensor.matmul(
                        out=out_ps[:],
                        lhsT=qsb[:, c, :],
                        rhs=ftT2_sb[:, c, :],
                        start=(c == 0), stop=(c == 3),
                    )
                return out_ps

            lg_ps = att_mm(qT2_sb, "lg_ps")
            # mask is mandatory: it also blanks the wrong-sample column half
            lg_sb = ps.tile([K44, N2], F32, name="lg", tag="lg")
            nc.vector.tensor_tensor(
                out=lg_sb[:], in0=lg_ps[:], in1=bm2mul, op=mybir.AluOpType.mult
            )
            nc.vector.tensor_tensor(
                out=lg_sb[:], in0=lg_sb[:], in1=bm2add, op=mybir.AluOpType.add
            )
            nmx = ps.tile([K44, 1], F32, tag="nmx")
            nc.vector.tensor_reduce(
                out=nmx[:], in_=lg_sb[:], axis=mybir.AxisListType.X,
                op=mybir.AluOpType.max, negate=True,
            )
            e_sb = ps.tile([K44, N2], F32, tag="e")
            ssum = ps.tile([K44, 1], F32, tag="ss")
            nc.scalar.activation(
                out=e_sb[:], in_=lg_sb[:],
                func=mybir.ActivationFunctionType.Exp,
                bias=nmx[:], scale=1.0, accum_out=ssum[:],
            )
            rs = ps.tile([K44, 1], F32, tag="rs")
            nc.vector.reciprocal(out=rs[:], in_=ssum[:])
            if not nm_ones:
                nc.vector.tensor_tensor(
                    out=rs[:], in0=rs[:], in1=nm2col, op=mybir.AluOpType.mult
                )
            find_sb = ps.tile([K44, N2], HALF, tag="find")
            nc.vector.tensor_scalar_mul(out=find_sb[:], in0=e_sb[:], scalar1=rs[:])

            A0_ps = att_mm(u0T2_sb, "A0_ps")
            A0_sb = ps.tile([K44, N2], HALF, tag="A0")
            nc.gpsimd.tensor_copy(out=A0_sb[:], in_=A0_ps[:])
            A1_ps = att_mm(u1T2_sb, "A1_ps")
            A1_sb = ps.tile([K44, N2], HALF, tag="A1")
            nc.gpsimd.tensor_copy(out=A1_sb[:], in_=A1_ps[:])

            # ---- gfT2 [128, R] and find2T init [128, K] (both samples) ----
            gfT2_ps = psum.tile([N2, R], F32, tag="ps", name="gfT2")
            for s in range(S):
                nc.tensor.matmul(
                    out=gfT2_ps[64 * s : 64 * s + N, :],
                    lhsT=find_sb[32 * s : 32 * s + K, N * s : N * (s + 1)],
                    rhs=GTm(s),
                    start=True, stop=True,
                    tile_position=(0, 64 * s),
                    skip_group_check=True,
                )
            gfT2_sb = ps.tile([N2, R], HALF, tag="gfT2sb")
            nc.gpsimd.tensor_copy(out=gfT2_sb[:], in_=gfT2_ps[:])

            f2T2_ps = psum.tile([N2, K], F32, tag="ps", name="f2T2")
            for s in range(S):
                nc.tensor.matmul(
                    out=f2T2_ps[64 * s : 64 * s + N, :],
                    lhsT=find_sb[32 * s : 32 * s + K, N * s : N * (s + 1)],
                    rhs=I128_sb[32 * s : 32 * s + K, 32 * s : 32 * s + K],
                    start=True, stop=True,
                    tile_position=(0, 64 * s),
                    skip_group_check=True,
                )

            # ---- edge attention ea [128, 12, 64]: tanh((A0+A1)/2) ----
            act_fn = (
                mybir.ActivationFunctionType.Tanh
                if TANH else mybir.ActivationFunctionType.Sigmoid
            )
            ea_all = ps.tile([N2, R, N], HALF, tag="ea")
            GR = R // 2
            for g in range(2):
                Bg = psum.tile([N2, GR, N], F32, tag="ps", name=f"B6_{g}")
                for s in range(S):
                    for i in range(GR):
                        r = GR * g + i
                        sel = I128_sb[
                            32 * s : 32 * s + K, 32 * s + r : 32 * s + r + 1
                        ].to_broadcast([K, N])
                        nc.tensor.matmul(
                            out=Bg[64 * s : 64 * s + N, i, :],
                            lhsT=sel,
                            rhs=A0_sb[32 * s : 32 * s + K, N * s : N * (s + 1)],
                            start=(i == 0), stop=False,
                            tile_position=(0, 64 * s),
                            skip_group_check=True,
                        )
                        nc.tensor.matmul(
                            out=Bg[64 * s : 64 * s + N, i, :],
                            lhsT=A1_sb[32 * s : 32 * s + K, N * s : N * (s + 1)],
                            rhs=sel,
                            start=False, stop=(i == GR - 1),
                            tile_position=(0, 64 * s),
                            skip_group_check=True,
                        )
                nc.scalar.activation(
                    out=ea_all[:, GR * g : GR * (g + 1), :], in_=Bg[:], func=act_fn
                )
            if not rm_ones:
                nc.vector.tensor_tensor(
                    out=ea_all[:],
                    in0=ea_all[:],
                    in1=rmask2[:, None, :].to_broadcast([N2, R, N]),
                    op=mybir.AluOpType.mult,
                )

            # ---- h-stage: hT2 [128, R] batched; transpose per sample ----
            hT2_ps = psum.tile([N2, R], F32, tag="ps", name="hT2")
            for s in range(S):
                for r in range(R):
                    nc.tensor.matmul(
                        out=hT2_ps[64 * s : 64 * s + N, r : r + 1],
                        lhsT=ea_all[64 * s : 64 * s + N, r, :],
                        rhs=gfT2_sb[64 * s : 64 * s + N, r : r + 1],
                        start=(r == 0), stop=(r == R - 1),
                        tile_position=(0, 64 * s),
                        skip_group_check=True,
                    )
            hT2_sb = ps.tile([N2, R], HALF, tag="hT2sb")
            if TANH:
                # sigmoid = 0.5*tanh + 0.5: h = 0.5*h_tanh + gs (gs pre-halved
                # and pre-broadcast down partitions host-side)
                nc.vector.scalar_tensor_tensor(
                    out=hT2_sb[:], in0=hT2_ps[:], scalar=0.5, in1=gs2,
                    op0=mybir.AluOpType.mult, op1=mybir.AluOpType.add,
                )
            else:
                nc.vector.tensor_copy(out=hT2_sb[:], in_=hT2_ps[:])

            h2_ps = psum.tile([K44, N], F32, tag="ps", name="h2")
            for s in range(S):
                nc.tensor.matmul(
                    out=h2_ps[32 * s : 32 * s + R, :],
                    lhsT=hT2_sb[64 * s : 64 * s + N, :],
                    rhs=I128_sb[64 * s : 64 * s + N, 64 * s : 64 * s + N],
                    start=True, stop=True,
                    tile_position=(0, 32 * s),
                    skip_group_check=True,
                )
            h2_sb = ps.tile([K44, N], HALF, tag="h2sb")
            for s in range(S):
                nc.gpsimd.tensor_copy(
                    out=h2_sb[32 * s : 32 * s + R, :],
                    in_=h2_ps[32 * s : 32 * s + R, :],
                )
            # scatter contribution in its own psum; summed on the drain (a
            # far-apart split accumulation group trips the scheduler)
            f2Tb_ps = psum.tile([N2, K], F32, tag="ps", name="f2Tb")
            for s in range(S):
                nc.tensor.matmul(
                    out=f2Tb_ps[64 * s : 64 * s + N, :],
                    lhsT=h2_sb[32 * s : 32 * s + R, :],
                    rhs=STm(s),
                    start=True, stop=True,
                    tile_position=(0, 64 * s),
                    skip_group_check=True,
                )

            # ---- final attention + mem ----
            f2sum = ps.tile([N2, K], F32, tag="f2sum")
            nc.vector.tensor_tensor(
                out=f2sum[:], in0=f2T2_ps[:], in1=f2Tb_ps[:],
                op=mybir.AluOpType.add,
            )
            fa2_sb = ps.tile([N2, 1], HALF, tag="fa2")
            nc.vector.tensor_reduce(
                out=fa2_sb[:], in_=f2sum[:], axis=mybir.AxisListType.X,
                op=mybir.AluOpType.max,
            )
            faT2_ps = psum.tile([1, N2], F32, tag="ps", name="faT2")
            nc.tensor.matmul(
                out=faT2_ps[:], lhsT=fa2_sb[:], rhs=I128_sb[:],
                start=True, stop=True,
            )
            faT2_sb = ps.tile([1, N2], HALF, tag="faT2sb")
            eng = [nc.vector, nc.vector]  # Pool cannot do free-axis reduces
            nr = [None, None]
            for s in range(S):
                e = eng[s]
                nr[s] = ps.tile([1, 1], F32, tag=f"nr{s}", name=f"nr{s}")
                e.tensor_reduce(
                    out=nr[s][:], in_=faT2_ps[:, N * s : N * (s + 1)],
                    axis=mybir.AxisListType.X, op=mybir.AluOpType.max,
                )
                e.tensor_scalar_max(out=nr[s][:], in0=nr[s][:], scalar1=1.0)
                e.reciprocal(out=nr[s][:], in_=nr[s][:])
                e.tensor_scalar_mul(
                    out=faT2_sb[:, N * s : N * (s + 1)],
                    in0=faT2_ps[:, N * s : N * (s + 1)], scalar1=nr[s][:],
                )
                if not bm_ones:
                    e.tensor_tensor(
                        out=faT2_sb[:, N * s : N * (s + 1)],
                        in0=faT2_sb[:, N * s : N * (s + 1)],
                        in1=sm[s][:1, _SM_FM : _SM_FM + N],
                        op=mybir.AluOpType.mult,
                    )
                    e.tensor_tensor(
                        out=faT2_sb[:, N * s : N * (s + 1)],
                        in0=faT2_sb[:, N * s : N * (s + 1)],
                        in1=sm[s][:1, _SM_FA : _SM_FA + N],
                        op=mybir.AluOpType.add,
                    )
            fabc2_ps = psum.tile([P, N2], F32, tag="ps", name="fabc2")
            nc.tensor.matmul(
                out=fabc2_ps[:], lhsT=ones_1xP[:], rhs=faT2_sb[:],
                start=True, stop=True,
            )
            fabc2_sb = ps.tile([P, N2], HALF, tag="fabc2sb")
            nc.gpsimd.tensor_copy(out=fabc2_sb[:], in_=fabc2_ps[:])

            # Pool does the broadcast multiplies, DVE the free-axis reduces
            # (Pool cannot reduce along X); reduce s0 overlaps multiply s1
            mem2r_sb = singles.tile([P, 16, S], HALF)
            wtmp = [None, None]
            for s in range(S):
                ns = slice(N * s, N * (s + 1))
                wtmp[s] = ps.tile([P, 16, N], HALF, tag=f"wtmp{s}", name=f"wtmp{s}")
                nc.gpsimd.tensor_tensor(
                    out=wtmp[s][:],
                    in0=visf2_mm[:, :, ns],
                    in1=fabc2_sb[:, None, ns].to_broadcast([P, 16, N]),
                    op=mybir.AluOpType.mult,
                )
            for s in range(S):
                with nc.allow_low_precision("fp16 mem rounding, matches cast"):
                    nc.vector.tensor_reduce(
                        out=mem2r_sb[:, :, s], in_=wtmp[s][:],
                        axis=mybir.AxisListType.X, op=mybir.AluOpType.add,
                    )

            # ---- W_out: weights stationary, 2-wide moving rhs ----
            o_ps = psum.tile([P, CBLK, S], F32, tag="ps", name="o_ps")
            for cb in range(CBLK):
                for c in range(16):
                    nc.tensor.matmul(
                        out=o_ps[:, cb, :],
                        lhsT=WoT_sb[:, cb // 4, c, P * (cb % 4) : P * (cb % 4 + 1)],
                        rhs=mem2r_sb[:, c, :],
                        start=(cb == 0 and c == 0),
                        stop=(cb == CBLK - 1 and c == 15),
                        skip_group_check=True,
                    )
            outT_sb = singles.tile([P, CBLK, S], HALF)
            nc.scalar.copy(out=outT_sb[:], in_=o_ps[:])
            if not bz:
                nc.vector.tensor_tensor(
                    out=outT_sb[:], in0=outT_sb[:],
                    in1=oscl_sb[:, _OS_BIAS : _OS_BIAS + CBLK, None].to_broadcast(
                        [P, CBLK, S]
                    ),
                    op=mybir.AluOpType.add,
                )
            oT_ps = psum.tile([S, CBLK, P], HALF, tag="ps", name="oT")
            for cb in range(CBLK):
                nc.tensor.transpose(
                    out=oT_ps[:, cb, :], in_=outT_sb[:, cb, :], identity=I128_sb[:]
                )
            outcat = singles.tile([S, DC], HALF)
            nc.vector.tensor_copy(
                out=outcat[:], in_=oT_ps[:].rearrange("s c p -> s (c p)")
            )
            nc.sync.dma_start(out=d_out[:], in_=outcat[:])

    nc.finalize()
    return nc


def _host_prep(inputs):
    node_rep = np.asarray(inputs["node_rep"], np.float32)
    relate_rep = np.asarray(inputs["relate_rep"], np.float32)
    relate_os = np.asarray(inputs["relate_os"])
    relate_mask = np.asarray(inputs["relate_mask"], np.float32)
    vision_feat = np.asarray(inputs["vision_feat"], np.float32)
    relation_mask = np.asarray(inputs["relation_mask"], np.float32)
    box_mask = np.asarray(inputs["box_mask"], np.float32)
    node_mask = np.asarray(inputs["node_mask"], np.float32)
    norm_w = np.asarray(inputs["norm_w"], np.float32)
    W_v = np.asarray(inputs["W_v"], np.float32)
    W_e = np.asarray(inputs["W_e"], np.float32)
    W_node = np.asarray(inputs["W_node"], np.float32)
    W_rel = np.asarray(inputs["W_rel"], np.float32)
    W_out = np.asarray(inputs["W_out"], np.float32)
    b_out = np.asarray(inputs["b_out"], np.float32)

    s_mean = np.float32(np.mean(norm_w))
    WvT = (W_v.T * s_mean).astype(np.float32)
    WnT = (W_node.T / np.float32(np.sqrt(DV))).astype(np.float32)
    WA0 = (W_rel.T @ W_e[:, :DV] / np.float32(np.sqrt(DE))).astype(np.float32)
    WA1 = (W_rel.T @ W_e[:, DV:] / np.float32(np.sqrt(DE))).astype(np.float32)
    WoT = np.ascontiguousarray(W_out.T)

    import ml_dtypes

    E4 = ml_dtypes.float8_e4m3
    fmax8 = np.float32(ml_dtypes.finfo(E4).max)

    def q8(a):
        s = np.float32(max(np.max(np.abs(a)), 1e-30) / fmax8)
        return (a / s).astype(E4), s

    Wv8, sv = q8(WvT)
    Wn8, tn = q8(WnT)
    WA08, t0 = q8(WA0)
    WA18, t1 = q8(WA1)

    rm_ones = bool(np.all(relation_mask == 1.0))
    tanh_form = rm_ones

    subj = relate_os[..., 1].astype(np.int64)
    obj = relate_os[..., 0].astype(np.int64)
    valid = (subj != -1).astype(np.float32)
    obj_c = np.clip(obj, 0, K - 1)
    subj_c = np.clip(subj, 0, K - 1)
    G = np.zeros((B, R, K), np.float32)
    STm = np.zeros((B, R, K), np.float32)
    bi = np.arange(B)[:, None]
    ri = np.arange(R)[None, :]
    G[bi, ri, obj_c] = valid * relate_mask
    STm[bi, ri, subj_c] = 1.0
    # tanh affine term: gs[b, r] = 0.5 * sum_k node_mask[b, k] * G[b, r, k]
    gs = 0.5 * np.einsum("bk,brk->br", node_mask, G).astype(np.float32)

    bmmul = (box_mask > 0).astype(np.float32)
    bmadd = (bmmul - 1.0) * np.float32(6e4)  # fp16-safe large negative
    famul = box_mask
    faadd = (1.0 - box_mask) * np.float32(1e-7)

    WvT_p = _pack(Wv8.astype(np.float32)).astype(E4)
    wq8_p = np.concatenate(
        [
            _pack(Wn8.astype(np.float32)),
            _pack(WA08.astype(np.float32)),
            _pack(WA18.astype(np.float32)),
        ],
        axis=1,
    ).astype(E4)
    WoT16_p = np.concatenate(
        [_pack(np.ascontiguousarray(WoT[:, 512 * h : 512 * (h + 1)])) for h in range(2)],
        axis=1,
    ).astype(np.float16)
    I128 = np.eye(P, dtype=np.float32)

    oscl = np.zeros((P, NSCL), np.float32)
    oscl[:, _OS_QN] = tn * sv
    half_f = np.float32(0.5) if tanh_form else np.float32(1.0)
    oscl[:, _OS_Q0] = t0 * sv * half_f
    oscl[:, _OS_Q1] = t1 * sv * half_f
    oscl[:, _OS_BIAS:] = b_out.reshape(CBLK, P).T

    in_maps = []
    for core in range(NCORES):
        b0 = S * core
        visf2 = np.concatenate(
            [_pack(vision_feat[b]).reshape(P, 16, N) for b in range(b0, b0 + S)],
            axis=2,
        ).reshape(P, -1)
        # reps in the padded 44-col layout (cols 32s..32s+12 per sample)
        nrep2 = np.zeros((P, 4, KP), np.float32)
        rrep2 = np.zeros((P, 4, KP), np.float32)
        for s in range(S):
            b = b0 + s
            nrep2[:, :, 32 * s : 32 * s + K] = _pack(
                np.ascontiguousarray(node_rep[b].T)
            ).reshape(P, 4, K)
            rrep2[:, :, 32 * s : 32 * s + R] = _pack(
                np.ascontiguousarray(relate_rep[b].T)
            ).reshape(P, 4, R)
        wcat_full = np.ascontiguousarray(
            np.concatenate([I128, nrep2.reshape(P, -1), rrep2.reshape(P, -1)], axis=1)
        )

        rest = np.zeros((P, REST_F), np.float32)
        # default: everything masked (mul=0, add=-6e4) including junk rows
        rest[:K44, _SH_BM : _SH_BM + N2] = 0.0
        rest[:K44, _SH_BA : _SH_BA + N2] = np.float32(-6e4)
        for s in range(S):
            b = b0 + s
            r0 = 32 * s
            c0n = N * s
            rest[r0 : r0 + K, _SH_BM + c0n : _SH_BM + c0n + N] = bmmul[b][None, :]
            rest[r0 : r0 + K, _SH_BA + c0n : _SH_BA + c0n + N] = bmadd[b][None, :]
            rest[r0 : r0 + K, _SH_NM] = node_mask[b]
            rest[64 * s : 64 * s + N, _SH_RM : _SH_RM + N] = relation_mask[b]
            rest[64 * s : 64 * s + N, _SH_GS : _SH_GS + R] = gs[b][None, :]
            c0 = SHARED_F + s * SMALLS_F
            rest[r0 : r0 + K, c0 + _SM_GT : c0 + _SM_GT + R] = G[b].T
            rest[r0 : r0 + R, c0 + _SM_ST : c0 + _SM_ST + K] = STm[b]
            rest[0, c0 + _SM_FM : c0 + _SM_FM + N] = famul[b]
            rest[0, c0 + _SM_FA : c0 + _SM_FA + N] = faadd[b]

        m = {
            "visf16": np.ascontiguousarray(visf2).astype(np.float16),
            "wcat16": wcat_full.astype(np.float16),
            "WvT8": WvT_p,
            "wq8": wq8_p,
            "resth": rest.astype(np.float16),
            "oscl": oscl,
            "WoT16": WoT16_p,
        }
        in_maps.append(m)
    return in_maps


def kernel(**inputs) -> np.ndarray:
    bm_ones = bool(np.all(np.asarray(inputs["box_mask"]) == 1.0))
    nm_ones = bool(np.all(np.asarray(inputs["node_mask"]) == 1.0))
    rm_ones = bool(np.all(np.asarray(inputs["relation_mask"]) == 1.0))
    bz = bool(np.all(np.asarray(inputs["b_out"]) == 0.0))
    key = ("nc", bm_ones, nm_ones, rm_ones, bz)
    if key not in _cache:
        _cache[key] = build_nc(bm_ones, nm_ones, rm_ones, bz)
    nc = _cache[key]
    in_maps = _host_prep(inputs)
    res = run_bass_kernel_spmd(nc, in_maps, core_ids=list(range(NCORES)))
    outs = [np.asarray(res.results[c]["out"], np.float32) for c in range(NCORES)]
    return np.concatenate(outs, axis=0)
